# revision 52
# baseline (speedup 1.0000x reference)
"""GQA attention kernel for 8 Trainium2 NeuronCores.

Sharding: core = (batch b, kv_group g), b in {0,1}, g in {0..3}.
Each core computes the 4 heads of one KV group for one batch and the
partial output projection for those heads; the host sums the 4 group
partials per batch.  Zero duplicated compute across cores.

All matmul operands are bf16 (fp32 PSUM accumulation).

v2 design — one merged PE stream (vs the v1 3-phase structure):
  The PE's pure-GEMM floor is ~1152 matmuls; v1 additionally spent 256
  matmuls (57us) on softmax denominators (ones-stationary partition
  reduction) and ran the output projection as a separate tail.  Here:
  - denominators: per attention step the exp'd P tile is accumulated on
    DVE (bf16 tensor_tensor add, 2x packed mode) into a per-pair acc;
    one ones-matmul per head per pair (16 total vs 256) reduces the
    partition dim.  1/sums = Exp(-Ln(sums)) on ACT: both functions live
    in the natural_log_exp_and_others table set (single load; the
    custom-DVE fast reciprocal does not compile on this walrus, and the
    stock InstReciprocal at ~6 cyc/elem would crowd the DVE).
  - the attention stream is ACT-bound at ~1.15us/step (exp of
    [128,2x512] at 1 elem/lane/cycle); the PE slack is filled by
    injecting Q-projection chains for the NEXT q-chunk (blocks 0-2) and
    output-projection chains (block 3) between the scores/AV matmuls.
  - stream order: K,V proj (all chunks) -> Q proj chunk 0 -> 128
    attention steps (8 head-pairs x 16 kv-tiles, consume lag 3) ->
    remaining Wo chains.  PSUM: scores rotation 4 banks + AV pair accum
    2 banks + shared proj/sums pool 2 banks = 8 exactly.
  - lag 3 (not 2) gives exp(s) ~4us before its consume deadline, so the
    ~2.6us boundary Ln/Exp burst on the ACT queue cannot stall the PE;
    the per-step acc-add is issued before the AV matmuls so the
    boundary ones-matmuls never wait on DVE.
  wk/wv host-prearranged partition-major; x/wq keep strided DMA
  patterns — an all-contiguous layout was measured (v1) to trigger a
  chip-wide ~20% DVFS clock drop.
"""

import numpy as np

# problem shape (hardcoded per contract)
B, S, E = 2, 2048, 2048
H, G, D = 16, 4, 128
R = H // G          # heads per kv group = 4
ST = S // 128       # 16 t-tiles
ET = E // 128       # 16 e-tiles
SC = S // 512       # 4 s-chunks
QC = S // 512       # 4 q-chunks

_cache = {}


def _split_multi_waits(nc, maxw=1):
    """Walrus in this container accepts only one sync-wait per
    instruction; move extra waits onto preceding same-engine NoOps."""
    from concourse import mybir

    n_split = 0
    for fn in nc.m.functions:
        for bb in fn.blocks:
            out = []
            changed = False
            for inst in bb.instructions:
                si = inst.sync_info
                waits = list(si.on_wait or []) if si is not None else []
                if len(waits) > maxw:
                    changed = True
                    n_split += 1
                    head, tail = waits[:-maxw], waits[-maxw:]
                    for j in range(0, len(head), maxw):
                        nop = mybir.InstNoOp(
                            name=f"{inst.name}-wsplit{j}", ins=[], outs=[]
                        )
                        nop.engine = inst.engine
                        nop.sync_info = mybir.SyncInfo(
                            on_wait=head[j : j + maxw], on_update=[]
                        )
                        out.append(nop)
                    si.on_wait = tail
                out.append(inst)
            if changed:
                bb.instructions = out
    return n_split


def _build_program():
    import contextlib

    import concourse.bass as bass
    import concourse.tile as tile
    from concourse import mybir

    BF16 = mybir.dt.bfloat16
    F32 = mybir.dt.float32
    Exp = mybir.ActivationFunctionType.Exp
    Ln = mybir.ActivationFunctionType.Ln
    Mult = mybir.AluOpType.mult
    Add = mybir.AluOpType.add

    nc = bass.Bass(target_bir_lowering=False)

    xT = nc.dram_tensor("xT", [E, S], BF16, kind="ExternalInput")
    wq = nc.dram_tensor("wq", [E, R * D], BF16, kind="ExternalInput")
    # wk/wv host-prearranged partition-major: their natural layout would
    # DMA as 256B rows at a fraction of peak
    wkh = nc.dram_tensor("wkh", [128, ET, D], BF16, kind="ExternalInput")
    wvh = nc.dram_tensor("wvh", [128, ET, D], BF16, kind="ExternalInput")
    wo = nc.dram_tensor("wo", [R * D, E], BF16, kind="ExternalInput")
    bqv = nc.dram_tensor("bqv", [R * D], F32, kind="ExternalInput")
    bkv = nc.dram_tensor("bkv", [D], F32, kind="ExternalInput")
    bvv = nc.dram_tensor("bvv", [D], F32, kind="ExternalInput")
    onesd = nc.dram_tensor("onesd", [128, 128], BF16, kind="ExternalInput")
    otd = nc.dram_tensor("ot", [E, S], BF16, kind="ExternalOutput")

    with tile.TileContext(nc) as tc:
        with contextlib.ExitStack() as ctx:
            consts = ctx.enter_context(tc.tile_pool(name="consts", bufs=1))
            big = ctx.enter_context(tc.tile_pool(name="big", bufs=1))

            bq_sb = consts.tile([128, R], F32)
            nc.gpsimd.dma_start(bq_sb, bqv.rearrange("(o p) -> p o", p=128))
            bk_sb = consts.tile([128, 1], F32)
            nc.gpsimd.dma_start(bk_sb, bkv.rearrange("(o p) -> p o", p=128))
            bv_sb = consts.tile([128, 1], F32)
            nc.gpsimd.dma_start(bv_sb, bvv.rearrange("(o p) -> p o", p=128))

            # host-provided (a gpsimd memset measured ~1us of preamble)
            ones = consts.tile([128, 128], BF16)
            nc.gpsimd.dma_start(ones, onesd[:, :])

            QT = big.tile([128, R, S], BF16)    # QT[d, h, q]
            KT = big.tile([128, S], BF16)       # KT[d, t]
            VT = big.tile([128, S], BF16)       # VT[d, t]
            V = big.tile([128, ST, D], BF16)    # V[t%128, tt, d]
            outT = big.tile([128, R, S], BF16)  # normalized attn out
            wo_sb = big.tile([128, R, E], BF16)
            wq_sb = big.tile([128, ET, R * D], BF16)
            wk_sb = big.tile([128, ET, D], BF16)
            wv_sb = big.tile([128, ET, D], BF16)
            # x chunks all resident (lifetimes overlap: chunk sc is read
            # again by the injected Q-proj of block sc-1)
            xts = [big.tile([128, ET, 512], BF16, name=f"xt{i}")
                   for i in range(SC)]

            # ---- input DMAs, balanced across the two HWDGE queues ----
            # Both queues carry x pieces (a dma_start blocks its issuing
            # engine for the whole transfer, so one queue alone delivers x
            # slower than the K/V chains consume it); weight pieces are
            # interleaved in consumption order.
            def _wchunk(dst, src_t, e0, e1):
                nc.scalar.dma_start(
                    dst[:, e0:e1],
                    src_t[e0 * 128 : e1 * 128, :].rearrange(
                        "(o p) m -> p o m", p=128
                    ),
                )

            def _xchunk(sc, k, eng):
                eng.dma_start(
                    xts[sc][:, k * 4 : (k + 1) * 4],
                    xT[k * 512 : (k + 1) * 512,
                       sc * 512 : (sc + 1) * 512].rearrange(
                        "(o p) m -> p o m", p=128
                    ),
                )

            def _wkv(w_sb, w_h, k):
                nc.scalar.dma_start(w_sb[:, 4 * k : 4 * k + 4],
                                    w_h[:, 4 * k : 4 * k + 4])

            # x rides the sync queue only: splitting x across both queues
            # measured ~20us WORSE (concurrent queues share DMA bandwidth,
            # doubling each transfer's latency, and the stream's first exp
            # then queues behind the ACT engine's remaining DMA issue)
            # (moving x3 to the scalar queue also measured worse — every
            # dual-queue x split loses to BW sharing between the queues)
            # the first wk/x pieces are split in half so the very first
            # K-chain matmuls wait on 2-etile rather than 4-etile transfers
            nc.scalar.dma_start(wk_sb[:, 0:2], wkh[:, 0:2])
            nc.sync.dma_start(
                xts[0][:, 0:2],
                xT[0:256, 0:512].rearrange("(o p) m -> p o m", p=128))
            nc.scalar.dma_start(wk_sb[:, 2:4], wkh[:, 2:4])
            nc.sync.dma_start(
                xts[0][:, 2:4],
                xT[256:512, 0:512].rearrange("(o p) m -> p o m", p=128))
            _wkv(wv_sb, wvh, 0)
            for k in range(1, 4):
                _wkv(wk_sb, wkh, k)
                _xchunk(0, k, nc.sync)
                _wkv(wv_sb, wvh, k)
            for k in range(4):
                _xchunk(1, k, nc.sync)
            for e0, e1 in ((0, 4), (4, 8), (8, 12), (12, 16)):
                _wchunk(wq_sb, wq, e0, e1)
            for sc in (2, 3):
                for k in range(4):
                    _xchunk(sc, k, nc.sync)

            ppool = ctx.enter_context(tc.tile_pool(name="probs", bufs=6))
            accpool = ctx.enter_context(tc.tile_pool(name="accs", bufs=2))
            upool = ctx.enter_context(tc.tile_pool(name="lns", bufs=3))
            rbpool = ctx.enter_context(tc.tile_pool(name="rbs", bufs=3))
            avspool = ctx.enter_context(tc.tile_pool(name="avsb", bufs=3))
            ostage = ctx.enter_context(tc.tile_pool(name="ostage", bufs=6))
            scpool = ctx.enter_context(
                tc.tile_pool(name="ps_sc", bufs=2, space="PSUM"))
            avpool = ctx.enter_context(
                tc.tile_pool(name="ps_av", bufs=1, space="PSUM"))
            # shared by projection chains AND per-pair sums matmuls;
            # scheduling keeps boundary steps free of chains
            projpool = ctx.enter_context(
                tc.tile_pool(name="ps_pj", bufs=2, space="PSUM"))

            # ---- solo phase: K,V all chunks, then Q chunk 0 ----
            # (running Q0 before KV2/KV3 measured worse: Q0 then stalls on
            # the wq pieces, which arrive after x1)
            def _kv_chunk(sc):
                cs = slice(sc * 512, (sc + 1) * 512)
                pk = projpool.tile([128, 512], F32, tag="proj",
                                   name=f"pk_{sc}")
                pv = projpool.tile([128, 512], F32, tag="proj",
                                   name=f"pv_{sc}")
                # interleave K/V per 4-e-tile group: tracks x-piece DMAs
                for g4 in range(4):
                    for e in range(g4 * 4, g4 * 4 + 4):
                        nc.tensor.matmul(pk, wk_sb[:, e], xts[sc][:, e],
                                         start=(e == 0), stop=(e == ET - 1))
                    for e in range(g4 * 4, g4 * 4 + 4):
                        nc.tensor.matmul(pv, wv_sb[:, e], xts[sc][:, e],
                                         start=(e == 0), stop=(e == ET - 1))
                # drains on DVE: the ACT queue is busy issuing DMAs in the
                # solo phase and would block these (PSUM frees) behind them
                nc.vector.tensor_scalar_add(KT[:, cs], pk, bk_sb[:, 0:1])
                nc.vector.tensor_scalar_add(VT[:, cs], pv, bv_sb[:, 0:1])
                for tt in range(sc * 4, sc * 4 + 4):
                    nc.sync.dma_start_transpose(
                        V[:, tt], VT[:, tt * 128 : (tt + 1) * 128]
                    )

            def _q0_chains():
                for h in range(R):
                    pq = projpool.tile([128, 512], F32, tag="proj",
                                       name=f"pq0_{h}")
                    for e in range(ET):
                        nc.tensor.matmul(
                            pq, wq_sb[:, e, h * 128 : (h + 1) * 128],
                            xts[0][:, e],
                            start=(e == 0), stop=(e == ET - 1))
                    nc.vector.tensor_scalar_add(QT[:, h, 0:512], pq,
                                                bq_sb[:, h : h + 1])

            for sc in range(SC):
                _kv_chunk(sc)
            _q0_chains()

            # wo is needed from block 3 on; SP queue drains x by ~30us
            nc.sync.dma_start(wo_sb, wo.rearrange("(o p) m -> p o m", p=128))

            # ---- injected work: Q-proj chains (blocks 0-2), Wo chains ----
            out_dma_n = [0]

            Cp = mybir.ActivationFunctionType.Copy

            def _emit_out_chunk(sc, et, eng, act_drain=False):
                po = projpool.tile([128, 512], F32, tag="proj",
                                   name=f"po_{sc}_{et}")
                for h in range(R):
                    nc.tensor.matmul(
                        po, wo_sb[:, h, et * 128 : (et + 1) * 128],
                        outT[:, h, sc * 512 : (sc + 1) * 512],
                        start=(h == 0), stop=(h == R - 1),
                    )
                st = ostage.tile([128, 512], BF16, tag="ost")
                if act_drain:
                    nc.scalar.activation(st, po, Cp)
                else:
                    nc.vector.tensor_copy(st, po)
                eng.dma_start(
                    otd[et * 128 : (et + 1) * 128, sc * 512 : (sc + 1) * 512],
                    st,
                )

            # inject[local_step] -> list of thunks, per block
            def _qproj_sched(qn):
                """Q-projection for chunk qn: 4 chains of 16 matmuls in
                windows clear of the pair-boundary steps {1,2,17,18}."""
                sched = {}
                windows = [(6, 11), (12, 17), (22, 27), (28, 31)]
                counts6 = [3, 3, 3, 3, 3, 1]
                counts5 = [4, 4, 4, 4]
                for h, (w0, w1) in enumerate(windows):
                    counts = counts6 if (w1 - w0) == 5 else counts5
                    pq = [None]

                    def _mk(h=h, qn=qn, pq=pq):
                        def _start():
                            pq[0] = projpool.tile([128, 512], F32, tag="proj",
                                                  name=f"pq{qn}_{h}")
                        return _start
                    e = [0]
                    start_fn = _mk()
                    for i, (ls, cnt) in enumerate(
                            zip(range(w0, w1 + 1), counts)):
                        def _mms(h=h, qn=qn, pq=pq, e=e, cnt=cnt,
                                 first=(i == 0), start_fn=start_fn):
                            if first:
                                start_fn()
                            for _ in range(cnt):
                                ei = e[0]
                                nc.tensor.matmul(
                                    pq[0],
                                    wq_sb[:, ei, h * 128 : (h + 1) * 128],
                                    xts[qn][:, ei],
                                    start=(ei == 0), stop=(ei == ET - 1),
                                )
                                e[0] += 1
                        sched.setdefault(ls, []).append(_mms)

                    def _drain(h=h, qn=qn, pq=pq):
                        nc.vector.tensor_scalar_add(
                            QT[:, h, qn * 512 : (qn + 1) * 512], pq[0],
                            bq_sb[:, h : h + 1])
                    sched.setdefault(w1, []).append(_drain)
                return sched

            def _wo_sched():
                """11 Wo chains for output chunk 0 inside block 3."""
                sched = {}
                slots = [(5, 6), (7, 8), (9, 10), (11, 12), (13, 14),
                         (15, 16), (22, 23), (24, 25), (26, 27), (28, 29),
                         (30, 31)]
                for et, (l0, l1) in enumerate(slots):
                    def _mk(et=et):
                        po_ref = [None]

                        def _first():
                            po_ref[0] = projpool.tile(
                                [128, 512], F32, tag="proj", name=f"po3_{et}")
                            for h in range(2):
                                nc.tensor.matmul(
                                    po_ref[0],
                                    wo_sb[:, h, et * 128 : (et + 1) * 128],
                                    outT[:, h, 0:512],
                                    start=(h == 0), stop=False,
                                )

                        def _second():
                            for h in range(2, R):
                                nc.tensor.matmul(
                                    po_ref[0],
                                    wo_sb[:, h, et * 128 : (et + 1) * 128],
                                    outT[:, h, 0:512],
                                    start=False, stop=(h == R - 1),
                                )
                            st = ostage.tile([128, 512], BF16, tag="ost")
                            nc.vector.tensor_copy(st, po_ref[0])
                            nc.sync.dma_start(
                                otd[et * 128 : (et + 1) * 128, 0:512], st)
                        return _first, _second
                    f1, f2 = _mk()
                    sched.setdefault(l0, []).append(f1)
                    sched.setdefault(l1, []).append(f2)
                return sched

            # ---- attention stream: 8 pairs x 16 t-tiles, lag-2 consume ----
            pairs = [(qc, hp) for qc in range(QC) for hp in range(R // 2)]
            NP = len(pairs)
            pts = {}
            av_cur = [None]
            acc_cur = [None]
            block_scheds = [_qproj_sched(1), _qproj_sched(2), _qproj_sched(3),
                            _wo_sched()]

            deferred = {}
            post_stream = []

            def _boundary(j, s):
                """End of pair j: denominators, reciprocal, normalize.
                The three ACT ops are staggered 3 steps apart so each
                0.7-1.2us injection amortizes into the per-step ACT slack
                instead of delaying exp(s) past the psc-reuse deadline."""
                if j == NP - 1:
                    # pairs 5/6's deferred recip work must precede pair 7's
                    # sums matmuls: they release the psum tiles those reuse
                    for fn in post_stream:
                        fn()
                    post_stream.clear()
                qc, hp = pairs[j]
                qs = slice(qc * 512, (qc + 1) * 512)
                hA, hB = 2 * hp, 2 * hp + 1
                acc = acc_cur[0]
                avp = av_cur[0]
                avs = avspool.tile([128, 2, 512], BF16, tag="avsb",
                                   name=f"avs_{j}")
                nc.vector.tensor_copy(avs, avp)  # frees av psum for next pair
                if j == NP - 1:
                    # no successor pair: put the sums in the freed av-pool
                    # tile so the first tail Wo chains don't wait on the
                    # proj pool behind the deferred Ln/Exp queue
                    sAB = avpool.tile([128, 2, 512], F32, tag="av",
                                      name="s_last")
                    sA, sB = sAB[:, 0], sAB[:, 1]
                else:
                    sA = projpool.tile([128, 512], F32, tag="proj",
                                       name=f"sA_{j}")
                    sB = projpool.tile([128, 512], F32, tag="proj",
                                       name=f"sB_{j}")
                nc.tensor.matmul(sA, ones, acc[:, 0], start=True, stop=True)
                nc.tensor.matmul(sB, ones, acc[:, 1], start=True, stop=True)
                u = upool.tile([128, 2, 512], F32, tag="ln", name=f"u_{j}")
                r = rbpool.tile([128, 2, 512], BF16, tag="rb", name=f"r_{j}")

                def _ln_a():
                    nc.scalar.activation(u[:, 0], sA, Ln)

                def _ln_b():
                    nc.scalar.activation(u[:, 1], sB, Ln)

                def _finish():
                    nc.scalar.activation(r, u, Exp, scale=-1.0)
                    nc.vector.tensor_tensor(outT[:, hA, qs], avs[:, 0],
                                            r[:, 0], Mult)
                    nc.vector.tensor_tensor(outT[:, hB, qs], avs[:, 1],
                                            r[:, 1], Mult)

                if j == NP - 1:
                    _ln_a(); _ln_b(); _finish()
                elif j in (5, 6):
                    # block 3 is ACT-bound; push these pairs' remaining
                    # recip work into the tail (ACT idle there, and their
                    # outputs are consumed 16-32us into the tail)
                    _ln_a()
                    post_stream.append(_ln_b)
                    post_stream.append(_finish)
                else:
                    _ln_a()
                    deferred.setdefault(s + 3, []).append(_ln_b)
                    deferred.setdefault(s + 6, []).append(_finish)

            for s in range(NP * ST + 3):
                for fn in deferred.pop(s, ()):
                    fn()
                c = s - 3
                # acc-add first: gives the pair-final add a step of lead
                # time so the boundary ones-matmuls never stall the PE
                if c >= 0:
                    jc, ttc = divmod(c, ST)
                    ptc = pts[c]
                    if ttc == 1:
                        acc_cur[0] = accpool.tile([128, 2, 512], BF16,
                                                  tag="acc", name=f"acc_{jc}")
                        nc.vector.tensor_tensor(acc_cur[0], pts[c - 1], ptc,
                                                Add)
                        del pts[c - 1]  # kept past its consume for this add
                    elif ttc > 1:
                        nc.vector.tensor_tensor(acc_cur[0], acc_cur[0], ptc,
                                                Add)
                if s < NP * ST:
                    j, tt = divmod(s, ST)
                    qc, hp = pairs[j]
                    qs = slice(qc * 512, (qc + 1) * 512)
                    hA, hB = 2 * hp, 2 * hp + 1
                    ks = KT[:, tt * 128 : (tt + 1) * 128]
                    # injected projection work FIRST: it has no exp
                    # dependency, so it executes inside any wait the scores
                    # matmul would impose on the PE FIFO (psc reuse waits
                    # exp(s-2) completion)
                    for fn in block_scheds[s // 32].get(s % 32, ()):
                        fn()
                    psc = scpool.tile([128, 2, 512], F32, tag="pss",
                                      name=f"psc_{s}")
                    # (a single N=1024 matmul for both heads fails the ISA
                    # check: a matmul output cannot span two PSUM banks)
                    nc.tensor.matmul(psc[:, 0], ks, QT[:, hA, qs],
                                     start=True, stop=True)
                    nc.tensor.matmul(psc[:, 1], ks, QT[:, hB, qs],
                                     start=True, stop=True)
                    pt = ppool.tile([128, 2, 512], BF16, tag="pt",
                                    name=f"pt_{s}")
                    nc.scalar.activation(pt, psc, Exp)
                    pts[s] = pt
                if c >= 0:
                    jc, ttc = divmod(c, ST)
                    # pt(c) with ttc==0 stays alive one more step: the
                    # ttc==1 acc-add reads it
                    ptc = pts[c] if ttc == 0 else pts.pop(c)
                    if ttc == 0:
                        av_cur[0] = avpool.tile([128, 2, 512], F32, tag="av",
                                                name=f"av_{jc}")
                    st_, sp_ = (ttc == 0), (ttc == ST - 1)
                    nc.tensor.matmul(av_cur[0][:, 0], V[:, ttc], ptc[:, 0],
                                     start=st_, stop=sp_)
                    nc.tensor.matmul(av_cur[0][:, 1], V[:, ttc], ptc[:, 1],
                                     start=st_, stop=sp_)
                    if ttc == ST - 1:
                        _boundary(jc, s)

            # flush any boundary ops scheduled past the stream end
            for key in sorted(deferred):
                for fn in deferred.pop(key):
                    fn()
            for fn in post_stream:
                fn()

            # ---- tail: remaining output-projection chains ----
            # 3-queue rotation measured best: two queues saturate on the
            # 53 transfers, and gpsimd-only-at-the-start also measured worse
            tail = [(0, et) for et in range(11, ET)]
            tail += [(sc, et) for sc in (1, 2, 3) for et in range(ET)]
            qs3 = (nc.sync, nc.scalar, nc.gpsimd)
            for i, (sc, et) in enumerate(tail):
                # last chains alternate drains ACT/DVE so the final
                # drain->DMA chains run on two engines concurrently
                _emit_out_chunk(sc, et, qs3[i % 3],
                                act_drain=(i >= len(tail) - 8 and i % 2 == 0))

    _split_multi_waits(nc)
    return nc


def _prepare(x, Wq, bq, Wk, bk, Wv, bv, Wo, bo):
    """Host-side sharding: build per-core input maps (bf16)."""
    import ml_dtypes

    bf16 = ml_dtypes.bfloat16
    x = np.asarray(x, dtype=np.float32)
    Wq = np.asarray(Wq, dtype=np.float32)
    bq = np.asarray(bq, dtype=np.float32)
    Wk = np.asarray(Wk, dtype=np.float32)
    bk = np.asarray(bk, dtype=np.float32)
    Wv = np.asarray(Wv, dtype=np.float32)
    bv = np.asarray(bv, dtype=np.float32)
    Wo = np.asarray(Wo, dtype=np.float32)

    isd = np.float32(1.0 / np.sqrt(D))

    xTs = [np.ascontiguousarray(x[b].T).astype(bf16) for b in range(B)]
    wqs = [
        np.ascontiguousarray(Wq[:, g * R * D : (g + 1) * R * D] * isd).astype(bf16)
        for g in range(G)
    ]
    def _pmajor(wmat):
        return np.ascontiguousarray(
            wmat.reshape(ET, 128, -1).transpose(1, 0, 2)).astype(bf16)

    wks = [_pmajor(Wk[:, g * D : (g + 1) * D]) for g in range(G)]
    wvs = [_pmajor(Wv[:, g * D : (g + 1) * D]) for g in range(G)]
    wos = [np.ascontiguousarray(Wo[g * R * D : (g + 1) * R * D, :]).astype(bf16)
           for g in range(G)]
    ones128 = np.ones((128, 128), dtype=bf16)
    in_maps = []
    for core in range(8):
        b, g = divmod(core, G)
        in_maps.append({
            "xT": xTs[b],
            "wq": wqs[g],
            "wkh": wks[g],
            "wvh": wvs[g],
            "wo": wos[g],
            "bqv": bq[g * R * D : (g + 1) * R * D] * isd,
            "bkv": bk[g * D : (g + 1) * D],
            "bvv": bv[g * D : (g + 1) * D],
            "onesd": ones128,
        })
    return in_maps


def _gather(results, bo):
    bo = np.asarray(bo, dtype=np.float32)
    out = np.empty((B, S, E), dtype=np.float32)
    for b in range(B):
        acc = results[b * G]["ot"].astype(np.float32)
        for g in range(1, G):
            acc += results[b * G + g]["ot"].astype(np.float32)
        out[b] = acc.T + bo
    return out


def kernel(x, Wq, bq, Wk, bk, Wv, bv, Wo, bo):
    from concourse.bass_utils import run_bass_kernel_spmd

    if "nc" not in _cache:
        _cache["nc"] = _build_program()
    nc = _cache["nc"]
    in_maps = _prepare(x, Wq, bq, Wk, bk, Wv, bv, Wo, bo)
    res = run_bass_kernel_spmd(nc, in_maps, core_ids=list(range(8)))
    return _gather(res.results, bo)
